# revision 6
# baseline (speedup 1.0000x reference)
"""Trainium2 Bass kernel for ContinuousGaussianVFE (segment reduce over voxel groups).

Approach
--------
Host side (numpy): bin points to voxel codes, stable-sort by code, find group
boundaries, and pack the sorted points into fixed-size per-tile slots
(128 groups per tile, up to 192 point slots per tile, zero-padded). Points are
laid out partition-major per 8-tile slab so every device DMA moves multi-KB
contiguous chunks per partition.

Device side (8 NeuronCores, SPMD, Tile framework): for each 128-group tile
  * build one-hot selection matrices S[point, group] = (seg[point] == iota)
    on the VectorEngine,
  * segment-sum all 90 feature columns + a count column with two accumulating
    TensorEngine matmuls (128- and 64-point partition chunks),
  * antipodal quaternion alignment: per-point dot with the group's reference
    quat (shipped per point), sign via ScalarEngine, applied in-place before
    the matmul,
  * means via per-partition scale (1/max(count,1)) during PSUM evacuation on
    the ScalarEngine; quaternion mean normalized on-chip,
  * voxel code -> (b,z,y,x) decode with integer shift/mask ops on GPSIMD.
Each core writes its contiguous range of group rows; host reassembles and pads
to the reference's fixed [N, ...] output shapes.
"""
import numpy as np

# ---------------- problem constants (fixed by the reference) ----------------
NPTS = 1_000_000
SCALE_XYZ = 1 << 23
SCALE_YZ = 1 << 14
SCALE_Z = 1 << 5
VOX = np.array([0.2, 0.2, 0.2], dtype=np.float32)
PC_MIN = np.array([-51.2, -51.2, -3.2], dtype=np.float32)

NCORES = 8
GP = 128           # groups per tile
PA, PB = 128, 64   # partition split of the point slots of one tile
SLAB = 8           # tiles per slab (DMA batching unit)
NCOL = 96          # 90 data + count-one + ref quat (4) + local seg id
OUTC = 90
# column map: 0:3 mu | 3:6 scale | 6:10 rot | 10:26 sem | 26:90 feat
#             90 count-one | 91:95 ref quat | 95 seg id

VOX_PAD_ROW = np.array([-1, 511, 511, 31], dtype=np.int32)


# ---------------------------------------------------------------------------
# The hardware allows at most one sync-wait per instruction; bacc.Bacc's
# finalize() pipeline (generate_event_semaphores) legalizes Tile's output.
# ---------------------------------------------------------------------------
def _make_tile_context_cls():
    from concourse.tile import TileContext

    return TileContext


# ---------------------------------------------------------------------------
# Device program
# ---------------------------------------------------------------------------
def build_program(slabs: int):
    import concourse.bacc as bacc
    import concourse.mybir as mybir

    f32 = mybir.dt.float32
    i32 = mybir.dt.int32
    Alu = mybir.AluOpType
    Act = mybir.ActivationFunctionType
    TileCtx = _make_tile_context_cls()

    nc = bacc.Bacc()
    xa = nc.declare_dram_parameter("xa", [slabs, PA, SLAB * NCOL], f32, isOutput=False)
    xb = nc.declare_dram_parameter("xb", [slabs, PB, SLAB * NCOL], f32, isOutput=False)
    uc = nc.declare_dram_parameter("uc", [slabs, SLAB, GP], i32, isOutput=False)
    io = nc.declare_dram_parameter("iota", [PA, GP], f32, isOutput=False)
    out = nc.declare_dram_parameter("out", [slabs, PA, SLAB * OUTC], f32, isOutput=True)
    vox = nc.declare_dram_parameter("vox", [4, slabs, SLAB, GP], i32, isOutput=True)

    with TileCtx(nc) as tc:
        with (
            tc.tile_pool(name="const", bufs=1) as const_p,
            tc.tile_pool(name="xa", bufs=3) as xa_p,
            tc.tile_pool(name="xb", bufs=3) as xb_p,
            tc.tile_pool(name="uc", bufs=2) as uc_p,
            tc.tile_pool(name="sel", bufs=6) as sel_p,
            tc.tile_pool(name="sm", bufs=4) as sm_p,
            tc.tile_pool(name="outp", bufs=3) as out_p,
            tc.tile_pool(name="voxp", bufs=2) as vox_p,
            tc.tile_pool(name="ps", bufs=8, space="PSUM") as ps_p,
        ):
            iota_t = const_p.tile([PA, GP], f32)
            nc.sync.dma_start(out=iota_t[:], in_=io[:, :])
            eps_t = const_p.tile([PA, 1], f32)
            nc.vector.memset(eps_t[:], 1e-8)

            for s in range(slabs):
                xa_t = xa_p.tile([PA, SLAB * NCOL], f32)
                nc.sync.dma_start(out=xa_t[:], in_=xa[s])
                xb_t = xb_p.tile([PB, SLAB * NCOL], f32)
                nc.sync.dma_start(out=xb_t[:], in_=xb[s])
                uc_t = uc_p.tile([SLAB, GP], i32)
                nc.sync.dma_start(out=uc_t[:], in_=uc[s])

                xa3 = xa_t[:].rearrange("p (t c) -> p t c", c=NCOL)
                xb3 = xb_t[:].rearrange("p (t c) -> p t c", c=NCOL)

                # ---- antipodal sign: per-point dot with group ref quat ----
                prod_a = sm_p.tile([PA, SLAB * 4], f32, tag="proda")
                pa3 = prod_a[:].rearrange("p (t c) -> p t c", c=4)
                nc.vector.tensor_tensor(
                    out=pa3, in0=xa3[:, :, 6:10], in1=xa3[:, :, 91:95], op=Alu.mult
                )
                dots_a = sm_p.tile([PA, SLAB], f32, tag="dota")
                nc.vector.tensor_reduce(
                    out=dots_a[:], in_=pa3, axis=mybir.AxisListType.X, op=Alu.add
                )
                sign_a = sm_p.tile([PA, SLAB], f32, tag="signa")
                nc.scalar.sign(sign_a[:], dots_a[:], bias=eps_t[:])
                nc.vector.tensor_tensor(
                    out=xa3[:, :, 6:10],
                    in0=xa3[:, :, 6:10],
                    in1=sign_a[:].rearrange("p (t o) -> p t o", o=1).to_broadcast(
                        [PA, SLAB, 4]
                    ),
                    op=Alu.mult,
                )

                prod_b = sm_p.tile([PB, SLAB * 4], f32, tag="prodb")
                pb3 = prod_b[:].rearrange("p (t c) -> p t c", c=4)
                nc.vector.tensor_tensor(
                    out=pb3, in0=xb3[:, :, 6:10], in1=xb3[:, :, 91:95], op=Alu.mult
                )
                dots_b = sm_p.tile([PB, SLAB], f32, tag="dotb")
                nc.vector.tensor_reduce(
                    out=dots_b[:], in_=pb3, axis=mybir.AxisListType.X, op=Alu.add
                )
                sign_b = sm_p.tile([PB, SLAB], f32, tag="signb")
                nc.scalar.sign(sign_b[:], dots_b[:], bias=eps_t[0:PB, :])
                nc.vector.tensor_tensor(
                    out=xb3[:, :, 6:10],
                    in0=xb3[:, :, 6:10],
                    in1=sign_b[:].rearrange("p (t o) -> p t o", o=1).to_broadcast(
                        [PB, SLAB, 4]
                    ),
                    op=Alu.mult,
                )

                # ---- per-tile segment sums via one-hot matmuls ----
                # Two halves of 4 tiles so at most 4 PSUM banks stay live
                # while the batched count-reciprocal completes.
                out_t = out_p.tile([PA, SLAB * OUTC], f32)
                cnt_t = sm_p.tile([PA, SLAB], f32, tag="cnt")
                rec_t = sm_p.tile([PA, SLAB], f32, tag="rec")
                HALF = SLAB // 2
                for h in range(2):
                    psums = []
                    for i in range(h * HALF, (h + 1) * HALF):
                        sa = sel_p.tile([PA, GP], f32, tag="sa")
                        nc.vector.tensor_scalar(
                            sa[:], iota_t[:], xa_t[:, i * NCOL + 95 : i * NCOL + 96],
                            None, Alu.is_equal,
                        )
                        sb = sel_p.tile([PB, GP], f32, tag="sb")
                        nc.vector.tensor_scalar(
                            sb[:], iota_t[0:PB, :], xb_t[:, i * NCOL + 95 : i * NCOL + 96],
                            None, Alu.is_equal,
                        )
                        ps = ps_p.tile([GP, 96], f32)
                        psums.append(ps)
                        nc.tensor.matmul(
                            ps[:, 0:91], sa[:], xa_t[:, i * NCOL : i * NCOL + 91],
                            start=True, stop=False,
                        )
                        nc.tensor.matmul(
                            ps[:, 0:91], sb[:], xb_t[0:PB, i * NCOL : i * NCOL + 91],
                            start=False, stop=True,
                        )
                        # counts -> SBUF (ScalarEngine copy; DVE can't be spared)
                        nc.scalar.activation(
                            cnt_t[:, i : i + 1], ps[:, 90:91], Act.Copy
                        )
                    # 1 / max(count, 1) for this half's 4 tiles at once
                    cslc = cnt_t[:, h * HALF : (h + 1) * HALF]
                    rslc = rec_t[:, h * HALF : (h + 1) * HALF]
                    nc.vector.tensor_scalar(cslc, cslc, 1.0, None, Alu.max)
                    nc.vector.reciprocal(rslc, cslc)
                    for k, i in enumerate(range(h * HALF, (h + 1) * HALF)):
                        nc.scalar.activation(
                            out_t[:, i * OUTC : i * OUTC + OUTC],
                            psums[k][:, 0:OUTC],
                            Act.Copy,
                            scale=rec_t[:, i : i + 1],
                        )

                # ---- normalize pooled quaternions ----
                o3 = out_t[:].rearrange("p (t c) -> p t c", c=OUTC)
                sq_t = sm_p.tile([PA, SLAB * 4], f32, tag="sq")
                sq3 = sq_t[:].rearrange("p (t c) -> p t c", c=4)
                nc.vector.tensor_tensor(
                    out=sq3, in0=o3[:, :, 6:10], in1=o3[:, :, 6:10], op=Alu.mult
                )
                nrm_t = sm_p.tile([PA, SLAB], f32, tag="nrm")
                nc.vector.tensor_reduce(
                    out=nrm_t[:], in_=sq3, axis=mybir.AxisListType.X, op=Alu.add
                )
                nc.scalar.sqrt(nrm_t[:], nrm_t[:])
                nc.vector.tensor_scalar(nrm_t[:], nrm_t[:], 1e-12, None, Alu.max)
                nrec_t = sm_p.tile([PA, SLAB], f32, tag="nrec")
                nc.vector.reciprocal(nrec_t[:], nrm_t[:])
                nc.vector.tensor_tensor(
                    out=o3[:, :, 6:10],
                    in0=o3[:, :, 6:10],
                    in1=nrec_t[:].rearrange("p (t o) -> p t o", o=1).to_broadcast(
                        [PA, SLAB, 4]
                    ),
                    op=Alu.mult,
                )
                nc.sync.dma_start(out=out[s], in_=out_t[:])

                # ---- voxel code decode (integer shift/mask ops) ----
                vx_t = vox_p.tile([SLAB, GP], i32, tag="vx")
                nc.vector.tensor_scalar(vx_t[:], uc_t[:], 31, None, Alu.bitwise_and)
                vy_t = vox_p.tile([SLAB, GP], i32, tag="vy")
                nc.vector.tensor_scalar(
                    vy_t[:], uc_t[:], 5, 511, Alu.arith_shift_right, Alu.bitwise_and
                )
                vz_t = vox_p.tile([SLAB, GP], i32, tag="vz")
                nc.vector.tensor_scalar(
                    vz_t[:], uc_t[:], 14, 511, Alu.arith_shift_right, Alu.bitwise_and
                )
                vb_t = vox_p.tile([SLAB, GP], i32, tag="vb")
                nc.vector.tensor_scalar(vb_t[:], uc_t[:], 23, None, Alu.arith_shift_right)
                nc.sync.dma_start(out=vox[0, s], in_=vb_t[:])
                nc.sync.dma_start(out=vox[1, s], in_=vz_t[:])
                nc.sync.dma_start(out=vox[2, s], in_=vy_t[:])
                nc.sync.dma_start(out=vox[3, s], in_=vx_t[:])

    nc.finalize()
    return nc


# ---------------------------------------------------------------------------
# Host-side prep / assembly
# ---------------------------------------------------------------------------
def host_prep(mu, scale, rotation, features, semantic, batch_idx, ncores=NCORES):
    n = mu.shape[0]
    idx = np.floor((mu - PC_MIN) / VOX).astype(np.int64)
    code = (
        batch_idx.astype(np.int64) * SCALE_XYZ
        + idx[:, 2] * SCALE_YZ
        + idx[:, 1] * SCALE_Z
        + idx[:, 0]
    )
    order = np.argsort(code, kind="stable")
    code_s = code[order]
    newg = np.empty(n, dtype=bool)
    newg[0] = True
    np.not_equal(code_s[1:], code_s[:-1], out=newg[1:])
    starts = np.flatnonzero(newg)
    m_used = len(starts)
    gid_s = np.cumsum(newg) - 1
    bounds = np.append(starts, n)

    t_total = -(-m_used // GP)
    tiles_per_core = -(-t_total // (ncores * SLAB)) * SLAB
    slabs = tiles_per_core // SLAB
    nt = ncores * tiles_per_core

    pf = PA + PB
    g0 = np.arange(t_total) * GP
    g1 = np.minimum(g0 + GP, m_used)
    tstart = np.full(nt, n, dtype=np.int64)
    tend = np.full(nt, n, dtype=np.int64)
    tstart[:t_total] = bounds[g0]
    tend[:t_total] = bounds[g1]
    span_max = int((tend - tstart).max())
    assert span_max <= pf, f"tile span {span_max} exceeds padded size {pf}"

    pidx = tstart[:, None] + np.arange(pf)[None, :]  # [nt, pf]
    valid = pidx < tend[:, None]
    pidx_c = np.minimum(pidx, n - 1)
    src = np.where(valid, order[pidx_c], n).ravel()
    gs = np.where(valid, gid_s[pidx_c], 0)
    seg = (gs - (np.arange(nt, dtype=np.int64) * GP)[:, None]).astype(np.float32)
    seg[~valid] = 0.0
    refsrc = np.where(valid, order[bounds[gs]], n).ravel()

    def ext(a):
        return np.vstack([a, np.zeros((1, a.shape[1]), np.float32)])

    big = np.empty((nt * pf, NCOL), dtype=np.float32)
    big[:, 0:3] = ext(mu)[src]
    big[:, 3:6] = ext(scale)[src]
    rot_e = ext(rotation)
    big[:, 6:10] = rot_e[src]
    big[:, 10:26] = ext(semantic)[src]
    big[:, 26:90] = ext(features)[src]
    big[:, 90] = valid.ravel().astype(np.float32)
    big[:, 91:95] = rot_e[refsrc]
    big[:, 95] = seg.ravel()
    big = big.reshape(nt, pf, NCOL)

    uc_all = np.full(nt * GP, -1, dtype=np.int32)
    uc_all[:m_used] = code_s[starts].astype(np.int32)
    uc_all = uc_all.reshape(nt, GP)

    iota = np.ascontiguousarray(
        np.broadcast_to(np.arange(GP, dtype=np.float32), (PA, GP))
    )

    in_maps = []
    for c in range(ncores):
        xc = big[c * tiles_per_core : (c + 1) * tiles_per_core]
        xa = (
            xc[:, :PA, :]
            .reshape(slabs, SLAB, PA, NCOL)
            .transpose(0, 2, 1, 3)
            .reshape(slabs, PA, SLAB * NCOL)
        )
        xb = (
            xc[:, PA:, :]
            .reshape(slabs, SLAB, PB, NCOL)
            .transpose(0, 2, 1, 3)
            .reshape(slabs, PB, SLAB * NCOL)
        )
        ucc = uc_all[c * tiles_per_core : (c + 1) * tiles_per_core].reshape(
            slabs, SLAB, GP
        )
        in_maps.append(
            {
                "xa": np.ascontiguousarray(xa),
                "xb": np.ascontiguousarray(xb),
                "uc": np.ascontiguousarray(ucc),
                "iota": iota,
            }
        )
    meta = {"slabs": slabs, "tiles_per_core": tiles_per_core, "n": n, "m_used": m_used}
    return in_maps, meta


def assemble(results, meta):
    n = meta["n"]
    slabs = meta["slabs"]
    rows = np.concatenate(
        [
            r["out"]
            .reshape(slabs, PA, SLAB, OUTC)
            .transpose(0, 2, 1, 3)
            .reshape(-1, OUTC)
            for r in results
        ]
    )
    voxr = np.concatenate(
        [r["vox"].transpose(1, 2, 3, 0).reshape(-1, 4) for r in results]
    )
    m_dev = min(rows.shape[0], n)

    def padf(cols):
        o = np.zeros((n, cols.shape[1]), dtype=np.float32)
        o[:m_dev] = cols[:m_dev]
        return o

    mu_out = padf(rows[:, 0:3])
    sc_out = padf(rows[:, 3:6])
    rot_out = padf(rows[:, 6:10])
    sem_out = padf(rows[:, 10:26])
    feat_out = padf(rows[:, 26:90])
    voxel_coords = np.empty((n, 4), dtype=np.int32)
    voxel_coords[:m_dev] = voxr[:m_dev]
    voxel_coords[m_dev:] = VOX_PAD_ROW
    return mu_out, sc_out, rot_out, feat_out, sem_out, voxel_coords


_PROGRAM_CACHE = {}


def kernel(mu, scale, rotation, features, semantic, batch_idx):
    from concourse.bass_utils import run_bass_kernel_spmd

    mu = np.asarray(mu, dtype=np.float32)
    scale = np.asarray(scale, dtype=np.float32)
    rotation = np.asarray(rotation, dtype=np.float32)
    features = np.asarray(features, dtype=np.float32)
    semantic = np.asarray(semantic, dtype=np.float32)
    batch_idx = np.asarray(batch_idx, dtype=np.int32)

    in_maps, meta = host_prep(mu, scale, rotation, features, semantic, batch_idx)
    slabs = meta["slabs"]
    if slabs not in _PROGRAM_CACHE:
        _PROGRAM_CACHE[slabs] = build_program(slabs)
    nc = _PROGRAM_CACHE[slabs]
    res = run_bass_kernel_spmd(nc, in_maps, list(range(NCORES)))
    return assemble(res.results, meta)


# revision 10
# speedup vs baseline: 1.9778x; 1.9778x over previous
"""Trainium2 Bass kernel for ContinuousGaussianVFE (segment reduce over voxel groups).

Approach
--------
Host (numpy): bin points to voxel codes, stable-sort, find group boundaries,
pack sorted points into fixed-size tiles (128 groups, up to 192 point slots),
split the 90 data columns into bf16 hi + bf16 lo parts (hi+lo matches fp32 to
~4e-6 relative), and lay everything out partition-major per 8-tile slab so
device DMAs move multi-KB contiguous chunks per partition.

Device (8 NeuronCores, SPMD, Tile): per 128-group tile
  * one-hot selection matrices S[point, group] = (seg == iota) in bf16 on the
    VectorEngine (exact 0/1 values),
  * segment sums via 4 accumulating TensorEngine matmuls (hi+lo x two
    partition chunks) at bf16 weight-load/stream rates into an fp32 PSUM tile,
  * antipodal quaternion alignment: fp32 per-point dot with the group's
    reference quat (shipped in a fp32 meta tensor), sign on the ScalarEngine,
    applied to the bf16 hi/lo quat columns in place,
  * means via ScalarEngine PSUM evacuation scaled by the host-computed
    1/max(count,1); pooled quaternions normalized on-chip,
  * voxel code decode (shift/mask) from int32 codes bit-packed into the meta
    tensor.
Host reassembles the per-core contiguous group rows and pads to [N, ...].
"""
import numpy as np
import ml_dtypes

BF16 = ml_dtypes.bfloat16

# ---------------- problem constants (fixed by the reference) ----------------
SCALE_XYZ = 1 << 23
SCALE_YZ = 1 << 14
SCALE_Z = 1 << 5
VOX = np.array([0.2, 0.2, 0.2], dtype=np.float32)
PC_MIN = np.array([-51.2, -51.2, -3.2], dtype=np.float32)

NCORES = 8
GP = 128           # groups per tile
PA, PB = 128, 64   # partition split of the point slots of one tile
SLAB = 8           # tiles per slab (DMA batching unit)
XCOL = 182         # per-tile columns in the bf16 stream: 90 hi | seg | 90 lo | pad
OUTC = 90
# data column map (within the 90): 0:3 mu | 3:6 scale | 6:10 rot | 10:26 sem | 26:90 feat
# meta fp32 tensor, per partition: [0:64) quat+ref for chunk A (8 per tile),
# [64:128) quat+ref chunk B (rows 0:64), [128:136) 1/max(cnt,1), [136:144) hi
# half of voxel code (code>>14, int bits), [144:152) low half (code&16383, int
# bits), [152:160) seg id chunk A, [160:168) seg id chunk B (rows 0:64).
# Codes are split so every int on the device stays < 2^24 (the DVE int path
# routes through fp32 and would round bigger values).
MQA, MQB, MREC, MUCH, MUCL, MSEGA, MSEGB, MCOL = 0, 64, 128, 136, 144, 152, 160, 168

VOX_PAD_ROW = np.array([-1, 511, 511, 31], dtype=np.int32)


# ---------------------------------------------------------------------------
# Device program
# ---------------------------------------------------------------------------
def build_program(slabs: int):
    import concourse.bacc as bacc
    import concourse.mybir as mybir
    from concourse.tile import TileContext

    f32 = mybir.dt.float32
    bf16 = mybir.dt.bfloat16
    i32 = mybir.dt.int32
    Alu = mybir.AluOpType
    Act = mybir.ActivationFunctionType

    nc = bacc.Bacc()
    xa = nc.declare_dram_parameter("xa", [slabs, PA, SLAB * XCOL], bf16, isOutput=False)
    xb = nc.declare_dram_parameter("xb", [slabs, PB, SLAB * XCOL], bf16, isOutput=False)
    mt = nc.declare_dram_parameter("meta", [slabs, PA, MCOL], f32, isOutput=False)
    io = nc.declare_dram_parameter("iota", [PA, GP], bf16, isOutput=False)
    out = nc.declare_dram_parameter("out", [slabs, PA, SLAB * OUTC], f32, isOutput=True)
    vox = nc.declare_dram_parameter("vox", [slabs, PA, 4 * SLAB], i32, isOutput=True)

    with TileContext(nc) as tc:
        with (
            tc.tile_pool(name="const", bufs=1) as const_p,
            tc.tile_pool(name="xa", bufs=3) as xa_p,
            tc.tile_pool(name="xb", bufs=3) as xb_p,
            tc.tile_pool(name="mt", bufs=3) as mt_p,
            tc.tile_pool(name="sel", bufs=8) as sel_p,
            tc.tile_pool(name="sm", bufs=4) as sm_p,
            tc.tile_pool(name="outp", bufs=3) as out_p,
            tc.tile_pool(name="voxp", bufs=3) as vox_p,
            tc.tile_pool(name="ps", bufs=8, space="PSUM") as ps_p,
        ):
            iota_t = const_p.tile([PA, GP], bf16)
            nc.sync.dma_start(out=iota_t[:], in_=io[:, :])
            eps_t = const_p.tile([PA, 1], f32)
            nc.vector.memset(eps_t[:], 1e-8)

            for s in range(slabs):
                xa_t = xa_p.tile([PA, SLAB * XCOL], bf16)
                nc.sync.dma_start(out=xa_t[:], in_=xa[s])
                xb_t = xb_p.tile([PB, SLAB * XCOL], bf16)
                nc.sync.dma_start(out=xb_t[:], in_=xb[s])
                mt_t = mt_p.tile([PA, MCOL], f32)
                nc.scalar.dma_start(out=mt_t[:], in_=mt[s])

                xa3 = xa_t[:].rearrange("p (t c) -> p t c", c=XCOL)
                xb3 = xb_t[:].rearrange("p (t c) -> p t c", c=XCOL)

                # ---- antipodal sign from fp32 quat/ref in meta ----
                mA = mt_t[:, MQA : MQA + 64].rearrange("p (t c) -> p t c", c=8)
                prod_a = sm_p.tile([PA, SLAB * 4], f32, tag="proda")
                pa3 = prod_a[:].rearrange("p (t c) -> p t c", c=4)
                nc.vector.tensor_tensor(
                    out=pa3, in0=mA[:, :, 0:4], in1=mA[:, :, 4:8], op=Alu.mult
                )
                dots_a = sm_p.tile([PA, SLAB], f32, tag="dota")
                nc.vector.tensor_reduce(
                    out=dots_a[:], in_=pa3, axis=mybir.AxisListType.X, op=Alu.add
                )
                sign_a = sm_p.tile([PA, SLAB], bf16, tag="signa")
                nc.scalar.sign(sign_a[:], dots_a[:], bias=eps_t[:])
                for cols in (slice(6, 10), slice(97, 101)):  # hi and lo quat cols
                    nc.vector.tensor_tensor(
                        out=xa3[:, :, cols],
                        in0=xa3[:, :, cols],
                        in1=sign_a[:]
                        .rearrange("p (t o) -> p t o", o=1)
                        .to_broadcast([PA, SLAB, 4]),
                        op=Alu.mult,
                    )

                mB = mt_t[0:PB, MQB : MQB + 64].rearrange("p (t c) -> p t c", c=8)
                prod_b = sm_p.tile([PB, SLAB * 4], f32, tag="prodb")
                pb3 = prod_b[:].rearrange("p (t c) -> p t c", c=4)
                nc.vector.tensor_tensor(
                    out=pb3, in0=mB[:, :, 0:4], in1=mB[:, :, 4:8], op=Alu.mult
                )
                dots_b = sm_p.tile([PB, SLAB], f32, tag="dotb")
                nc.vector.tensor_reduce(
                    out=dots_b[:], in_=pb3, axis=mybir.AxisListType.X, op=Alu.add
                )
                sign_b = sm_p.tile([PB, SLAB], bf16, tag="signb")
                nc.scalar.sign(sign_b[:], dots_b[:], bias=eps_t[0:PB, :])
                for cols in (slice(6, 10), slice(97, 101)):
                    nc.vector.tensor_tensor(
                        out=xb3[:, :, cols],
                        in0=xb3[:, :, cols],
                        in1=sign_b[:]
                        .rearrange("p (t o) -> p t o", o=1)
                        .to_broadcast([PB, SLAB, 4]),
                        op=Alu.mult,
                    )

                # ---- per-tile segment sums via one-hot matmuls (bf16 hi+lo) ----
                out_t = out_p.tile([PA, SLAB * OUTC], f32)
                for i in range(SLAB):
                    sa = sel_p.tile([PA, GP], bf16, tag="sa")
                    nc.vector.tensor_scalar(
                        sa[:], iota_t[:], mt_t[:, MSEGA + i : MSEGA + i + 1],
                        None, Alu.is_equal,
                    )
                    sb = sel_p.tile([PB, GP], bf16, tag="sb")
                    nc.vector.tensor_scalar(
                        sb[:], iota_t[0:PB, :], mt_t[0:PB, MSEGB + i : MSEGB + i + 1],
                        None, Alu.is_equal,
                    )
                    ps = ps_p.tile([GP, OUTC], f32)
                    nc.tensor.matmul(
                        ps[:], sa[:], xa_t[:, i * XCOL : i * XCOL + 90],
                        start=True, stop=False,
                    )
                    nc.tensor.matmul(
                        ps[:], sa[:], xa_t[:, i * XCOL + 91 : i * XCOL + 181],
                        start=False, stop=False,
                    )
                    nc.tensor.matmul(
                        ps[:], sb[:], xb_t[0:PB, i * XCOL : i * XCOL + 90],
                        start=False, stop=False,
                    )
                    nc.tensor.matmul(
                        ps[:], sb[:], xb_t[0:PB, i * XCOL + 91 : i * XCOL + 181],
                        start=False, stop=True,
                    )
                    nc.scalar.activation(
                        out_t[:, i * OUTC : i * OUTC + OUTC],
                        ps[:],
                        Act.Copy,
                        scale=mt_t[:, MREC + i : MREC + i + 1],
                    )

                # ---- normalize pooled quaternions ----
                o3 = out_t[:].rearrange("p (t c) -> p t c", c=OUTC)
                sq_t = sm_p.tile([PA, SLAB * 4], f32, tag="sq")
                sq3 = sq_t[:].rearrange("p (t c) -> p t c", c=4)
                nc.vector.tensor_tensor(
                    out=sq3, in0=o3[:, :, 6:10], in1=o3[:, :, 6:10], op=Alu.mult
                )
                nrm_t = sm_p.tile([PA, SLAB], f32, tag="nrm")
                nc.vector.tensor_reduce(
                    out=nrm_t[:], in_=sq3, axis=mybir.AxisListType.X, op=Alu.add
                )
                nc.scalar.sqrt(nrm_t[:], nrm_t[:])
                nc.vector.tensor_scalar(nrm_t[:], nrm_t[:], 1e-12, None, Alu.max)
                nrec_t = sm_p.tile([PA, SLAB], f32, tag="nrec")
                nc.vector.reciprocal(nrec_t[:], nrm_t[:])
                nc.vector.tensor_tensor(
                    out=o3[:, :, 6:10],
                    in0=o3[:, :, 6:10],
                    in1=nrec_t[:]
                    .rearrange("p (t o) -> p t o", o=1)
                    .to_broadcast([PA, SLAB, 4]),
                    op=Alu.mult,
                )
                nc.sync.dma_start(out=out[s], in_=out_t[:])

                # ---- voxel code decode (shift/mask on small int halves) ----
                uch = mt_t[:, MUCH : MUCH + SLAB].bitcast(i32)
                ucl = mt_t[:, MUCL : MUCL + SLAB].bitcast(i32)
                vox_t = vox_p.tile([PA, 4 * SLAB], i32)
                nc.vector.tensor_scalar(
                    vox_t[:, 0:SLAB], uch, 9, None, Alu.arith_shift_right
                )
                nc.vector.tensor_scalar(
                    vox_t[:, SLAB : 2 * SLAB], uch, 511, None, Alu.bitwise_and
                )
                nc.vector.tensor_scalar(
                    vox_t[:, 2 * SLAB : 3 * SLAB], ucl, 5, None, Alu.arith_shift_right
                )
                nc.vector.tensor_scalar(
                    vox_t[:, 3 * SLAB : 4 * SLAB], ucl, 31, None, Alu.bitwise_and
                )
                nc.scalar.dma_start(out=vox[s], in_=vox_t[:])

    nc.finalize()
    return nc


# ---------------------------------------------------------------------------
# Host-side prep / assembly
# ---------------------------------------------------------------------------
def host_prep(mu, scale, rotation, features, semantic, batch_idx, ncores=NCORES):
    n = mu.shape[0]
    idx = np.floor((mu - PC_MIN) / VOX).astype(np.int64)
    code = (
        batch_idx.astype(np.int64) * SCALE_XYZ
        + idx[:, 2] * SCALE_YZ
        + idx[:, 1] * SCALE_Z
        + idx[:, 0]
    )
    order = np.argsort(code, kind="stable")
    code_s = code[order]
    newg = np.empty(n, dtype=bool)
    newg[0] = True
    np.not_equal(code_s[1:], code_s[:-1], out=newg[1:])
    starts = np.flatnonzero(newg)
    m_used = len(starts)
    gid_s = np.cumsum(newg) - 1
    bounds = np.append(starts, n)

    t_total = -(-m_used // GP)
    tiles_per_core = -(-t_total // (ncores * SLAB)) * SLAB
    slabs = tiles_per_core // SLAB
    nt = ncores * tiles_per_core

    pf = PA + PB
    g0 = np.arange(t_total) * GP
    g1 = np.minimum(g0 + GP, m_used)
    tstart = np.full(nt, n, dtype=np.int64)
    tend = np.full(nt, n, dtype=np.int64)
    tstart[:t_total] = bounds[g0]
    tend[:t_total] = bounds[g1]
    span_max = int((tend - tstart).max())
    assert span_max <= pf, f"tile span {span_max} exceeds padded size {pf}"

    pidx = tstart[:, None] + np.arange(pf)[None, :]  # [nt, pf]
    valid = pidx < tend[:, None]
    pidx_c = np.minimum(pidx, n - 1)
    src = np.where(valid, order[pidx_c], n).ravel()
    gs = np.where(valid, gid_s[pidx_c], 0)
    seg = (gs - (np.arange(nt, dtype=np.int64) * GP)[:, None]).astype(np.float32)
    seg[~valid] = 0.0
    refsrc = np.where(valid, order[bounds[gs]], n).ravel()

    def ext(a):
        return np.vstack([a, np.zeros((1, a.shape[1]), np.float32)])

    data = np.empty((nt * pf, OUTC), dtype=np.float32)
    data[:, 0:3] = ext(mu)[src]
    data[:, 3:6] = ext(scale)[src]
    rot_e = ext(rotation)
    data[:, 6:10] = rot_e[src]
    data[:, 10:26] = ext(semantic)[src]
    data[:, 26:90] = ext(features)[src]
    hi = data.astype(BF16)
    lo = (data - hi.astype(np.float32)).astype(BF16)

    xcols = np.zeros((nt * pf, XCOL), dtype=BF16)
    xcols[:, 0:90] = hi
    xcols[:, 91:181] = lo
    xcols = xcols.reshape(nt, pf, XCOL)
    seg3 = seg.reshape(nt, pf)

    # per-group metadata
    counts = np.zeros(nt * GP, dtype=np.float32)
    counts[:m_used] = np.diff(bounds).astype(np.float32)
    rec_all = (1.0 / np.maximum(counts, 1.0)).reshape(nt, GP)
    # codes split into two small halves (device ints must stay < 2^24); pad
    # groups ship 0 and the host overwrites their voxel rows after the run
    uc_full = np.zeros(nt * GP, dtype=np.int32)
    uc_full[:m_used] = code_s[starts].astype(np.int32)
    uch_all = (uc_full >> 14).reshape(nt, GP)
    ucl_all = (uc_full & 16383).reshape(nt, GP)

    iota = np.ascontiguousarray(
        np.broadcast_to(np.arange(GP, dtype=np.float32), (PA, GP)).astype(BF16)
    )

    # quat + ref fp32 per point, tile-slot layout
    qr = np.empty((nt * pf, 8), dtype=np.float32)
    qr[:, 0:4] = rot_e[src]
    qr[:, 4:8] = rot_e[refsrc]
    qr = qr.reshape(nt, pf, 8)

    in_maps = []
    for c in range(ncores):
        sl = slice(c * tiles_per_core, (c + 1) * tiles_per_core)
        xc = xcols[sl]
        xa = (
            xc[:, :PA, :]
            .reshape(slabs, SLAB, PA, XCOL)
            .transpose(0, 2, 1, 3)
            .reshape(slabs, PA, SLAB * XCOL)
        )
        xb = (
            xc[:, PA:, :]
            .reshape(slabs, SLAB, PB, XCOL)
            .transpose(0, 2, 1, 3)
            .reshape(slabs, PB, SLAB * XCOL)
        )
        meta = np.zeros((slabs, PA, MCOL), dtype=np.float32)
        qc = qr[sl]
        meta[:, :, MQA : MQA + 64] = (
            qc[:, :PA, :]
            .reshape(slabs, SLAB, PA, 8)
            .transpose(0, 2, 1, 3)
            .reshape(slabs, PA, 64)
        )
        meta[:, :PB, MQB : MQB + 64] = (
            qc[:, PA:, :]
            .reshape(slabs, SLAB, PB, 8)
            .transpose(0, 2, 1, 3)
            .reshape(slabs, PB, 64)
        )
        meta[:, :, MREC : MREC + SLAB] = (
            rec_all[sl].reshape(slabs, SLAB, GP).transpose(0, 2, 1)
        )
        meta[:, :, MUCH : MUCH + SLAB] = np.ascontiguousarray(
            uch_all[sl].reshape(slabs, SLAB, GP).transpose(0, 2, 1)
        ).view(np.float32)
        meta[:, :, MUCL : MUCL + SLAB] = np.ascontiguousarray(
            ucl_all[sl].reshape(slabs, SLAB, GP).transpose(0, 2, 1)
        ).view(np.float32)
        meta[:, :, MSEGA : MSEGA + SLAB] = (
            seg3[sl, :PA].reshape(slabs, SLAB, PA).transpose(0, 2, 1)
        )
        meta[:, :PB, MSEGB : MSEGB + SLAB] = (
            seg3[sl, PA:].reshape(slabs, SLAB, PB).transpose(0, 2, 1)
        )
        in_maps.append(
            {
                "xa": np.ascontiguousarray(xa),
                "xb": np.ascontiguousarray(xb),
                "meta": meta,
                "iota": iota,
            }
        )
    meta_info = {
        "slabs": slabs,
        "tiles_per_core": tiles_per_core,
        "n": n,
        "m_used": m_used,
    }
    return in_maps, meta_info


def assemble(results, meta):
    n = meta["n"]
    slabs = meta["slabs"]
    rows = np.concatenate(
        [
            r["out"]
            .reshape(slabs, PA, SLAB, OUTC)
            .transpose(0, 2, 1, 3)
            .reshape(-1, OUTC)
            for r in results
        ]
    )
    voxr = np.concatenate(
        [
            r["vox"]
            .reshape(slabs, PA, 4, SLAB)
            .transpose(0, 3, 1, 2)
            .reshape(-1, 4)
            for r in results
        ]
    )
    m_dev = min(rows.shape[0], n)

    def padf(cols):
        o = np.zeros((n, cols.shape[1]), dtype=np.float32)
        o[:m_dev] = cols[:m_dev]
        return o

    mu_out = padf(rows[:, 0:3])
    sc_out = padf(rows[:, 3:6])
    rot_out = padf(rows[:, 6:10])
    sem_out = padf(rows[:, 10:26])
    feat_out = padf(rows[:, 26:90])
    m_real = min(meta["m_used"], n)
    voxel_coords = np.empty((n, 4), dtype=np.int32)
    voxel_coords[:m_real] = voxr[:m_real]
    voxel_coords[m_real:] = VOX_PAD_ROW
    return mu_out, sc_out, rot_out, feat_out, sem_out, voxel_coords


_PROGRAM_CACHE = {}


def kernel(mu, scale, rotation, features, semantic, batch_idx):
    from concourse.bass_utils import run_bass_kernel_spmd

    mu = np.asarray(mu, dtype=np.float32)
    scale = np.asarray(scale, dtype=np.float32)
    rotation = np.asarray(rotation, dtype=np.float32)
    features = np.asarray(features, dtype=np.float32)
    semantic = np.asarray(semantic, dtype=np.float32)
    batch_idx = np.asarray(batch_idx, dtype=np.int32)

    in_maps, meta = host_prep(mu, scale, rotation, features, semantic, batch_idx)
    slabs = meta["slabs"]
    if slabs not in _PROGRAM_CACHE:
        _PROGRAM_CACHE[slabs] = build_program(slabs)
    nc = _PROGRAM_CACHE[slabs]
    res = run_bass_kernel_spmd(nc, in_maps, list(range(NCORES)))
    return assemble(res.results, meta)


# revision 11
# speedup vs baseline: 2.0150x; 1.0188x over previous
"""Trainium2 Bass kernel for ContinuousGaussianVFE (segment reduce over voxel groups).

Approach
--------
Host (numpy): bin points to voxel codes, stable-sort, find group boundaries,
pack sorted points into fixed-size tiles (128 groups, up to 192 point slots),
split the 90 data columns into bf16 hi + bf16 lo parts (hi+lo matches fp32 to
~4e-6 relative), and lay everything out partition-major per 8-tile slab so
device DMAs move multi-KB contiguous chunks per partition.

Device (8 NeuronCores, SPMD, Tile): per 128-group tile
  * one-hot selection matrices S[point, group] = (seg == iota) in bf16 on the
    VectorEngine (exact 0/1 values),
  * segment sums via 4 accumulating TensorEngine matmuls (hi+lo x two
    partition chunks) at bf16 weight-load/stream rates into an fp32 PSUM tile,
  * antipodal quaternion alignment: fp32 per-point dot with the group's
    reference quat (shipped in a fp32 meta tensor), sign on the ScalarEngine,
    applied to the bf16 hi/lo quat columns in place,
  * means via ScalarEngine PSUM evacuation scaled by the host-computed
    1/max(count,1); pooled quaternions normalized on-chip,
  * voxel code decode (shift/mask) from int32 codes bit-packed into the meta
    tensor.
Host reassembles the per-core contiguous group rows and pads to [N, ...].
"""
import numpy as np
import ml_dtypes

BF16 = ml_dtypes.bfloat16

# ---------------- problem constants (fixed by the reference) ----------------
SCALE_XYZ = 1 << 23
SCALE_YZ = 1 << 14
SCALE_Z = 1 << 5
VOX = np.array([0.2, 0.2, 0.2], dtype=np.float32)
PC_MIN = np.array([-51.2, -51.2, -3.2], dtype=np.float32)

NCORES = 8
GP = 128           # groups per tile
PA, PB = 128, 64   # partition split of the point slots of one tile
SLAB = 8           # tiles per slab (DMA batching unit)
XCOL = 182         # per-tile columns in the bf16 stream: 90 hi | seg | 90 lo | pad
OUTC = 90
# data column map (within the 90): 0:3 mu | 3:6 scale | 6:10 rot | 10:26 sem | 26:90 feat
# meta fp32 tensor, per partition: [0:64) quat+ref for chunk A (8 per tile),
# [64:128) quat+ref chunk B (rows 0:64), [128:136) 1/max(cnt,1), [136:144) hi
# half of voxel code (code>>14, int bits), [144:152) low half (code&16383, int
# bits), [152:160) seg id chunk A, [160:168) seg id chunk B (rows 0:64).
# Codes are split so every int on the device stays < 2^24 (the DVE int path
# routes through fp32 and would round bigger values).
MQA, MQB, MREC, MUCH, MUCL, MSEGA, MSEGB, MCOL = 0, 64, 128, 136, 144, 152, 160, 168

VOX_PAD_ROW = np.array([-1, 511, 511, 31], dtype=np.int32)


# ---------------------------------------------------------------------------
# Device program
# ---------------------------------------------------------------------------
def build_program(slabs: int):
    import concourse.bacc as bacc
    import concourse.mybir as mybir
    from concourse.tile import TileContext

    f32 = mybir.dt.float32
    bf16 = mybir.dt.bfloat16
    i32 = mybir.dt.int32
    Alu = mybir.AluOpType
    Act = mybir.ActivationFunctionType

    nc = bacc.Bacc()
    xa = nc.declare_dram_parameter("xa", [slabs, PA, SLAB * XCOL], bf16, isOutput=False)
    xb = nc.declare_dram_parameter("xb", [slabs, PB, SLAB * XCOL], bf16, isOutput=False)
    mt = nc.declare_dram_parameter("meta", [slabs, PA, MCOL], f32, isOutput=False)
    io = nc.declare_dram_parameter("iota", [PA, GP], bf16, isOutput=False)
    out = nc.declare_dram_parameter("out", [slabs, PA, SLAB * OUTC], f32, isOutput=True)
    vox = nc.declare_dram_parameter("vox", [slabs, PA, 4 * SLAB], i32, isOutput=True)

    with TileContext(nc) as tc:
        with (
            tc.tile_pool(name="const", bufs=1) as const_p,
            tc.tile_pool(name="xa", bufs=4) as xa_p,
            tc.tile_pool(name="xb", bufs=4) as xb_p,
            tc.tile_pool(name="mt", bufs=4) as mt_p,
            tc.tile_pool(name="sel", bufs=12) as sel_p,
            tc.tile_pool(name="sm", bufs=6) as sm_p,
            tc.tile_pool(name="outp", bufs=4) as out_p,
            tc.tile_pool(name="voxp", bufs=3) as vox_p,
            tc.tile_pool(name="ps", bufs=8, space="PSUM") as ps_p,
        ):
            iota_t = const_p.tile([PA, GP], bf16)
            nc.sync.dma_start(out=iota_t[:], in_=io[:, :])
            eps_t = const_p.tile([PA, 1], f32)
            nc.vector.memset(eps_t[:], 1e-8)

            for s in range(slabs):
                xa_t = xa_p.tile([PA, SLAB * XCOL], bf16)
                nc.sync.dma_start(out=xa_t[:], in_=xa[s])
                xb_t = xb_p.tile([PB, SLAB * XCOL], bf16)
                nc.sync.dma_start(out=xb_t[:], in_=xb[s])
                mt_t = mt_p.tile([PA, MCOL], f32)
                nc.scalar.dma_start(out=mt_t[:], in_=mt[s])

                xa3 = xa_t[:].rearrange("p (t c) -> p t c", c=XCOL)
                xb3 = xb_t[:].rearrange("p (t c) -> p t c", c=XCOL)

                # ---- antipodal sign from fp32 quat/ref in meta ----
                mA = mt_t[:, MQA : MQA + 64].rearrange("p (t c) -> p t c", c=8)
                prod_a = sm_p.tile([PA, SLAB * 4], f32, tag="proda")
                pa3 = prod_a[:].rearrange("p (t c) -> p t c", c=4)
                nc.vector.tensor_tensor(
                    out=pa3, in0=mA[:, :, 0:4], in1=mA[:, :, 4:8], op=Alu.mult
                )
                dots_a = sm_p.tile([PA, SLAB], f32, tag="dota")
                nc.vector.tensor_reduce(
                    out=dots_a[:], in_=pa3, axis=mybir.AxisListType.X, op=Alu.add
                )
                sign_a = sm_p.tile([PA, SLAB], bf16, tag="signa")
                nc.scalar.sign(sign_a[:], dots_a[:], bias=eps_t[:])
                for cols in (slice(6, 10), slice(97, 101)):  # hi and lo quat cols
                    nc.vector.tensor_tensor(
                        out=xa3[:, :, cols],
                        in0=xa3[:, :, cols],
                        in1=sign_a[:]
                        .rearrange("p (t o) -> p t o", o=1)
                        .to_broadcast([PA, SLAB, 4]),
                        op=Alu.mult,
                    )

                mB = mt_t[0:PB, MQB : MQB + 64].rearrange("p (t c) -> p t c", c=8)
                prod_b = sm_p.tile([PB, SLAB * 4], f32, tag="prodb")
                pb3 = prod_b[:].rearrange("p (t c) -> p t c", c=4)
                nc.vector.tensor_tensor(
                    out=pb3, in0=mB[:, :, 0:4], in1=mB[:, :, 4:8], op=Alu.mult
                )
                dots_b = sm_p.tile([PB, SLAB], f32, tag="dotb")
                nc.vector.tensor_reduce(
                    out=dots_b[:], in_=pb3, axis=mybir.AxisListType.X, op=Alu.add
                )
                sign_b = sm_p.tile([PB, SLAB], bf16, tag="signb")
                nc.scalar.sign(sign_b[:], dots_b[:], bias=eps_t[0:PB, :])
                for cols in (slice(6, 10), slice(97, 101)):
                    nc.vector.tensor_tensor(
                        out=xb3[:, :, cols],
                        in0=xb3[:, :, cols],
                        in1=sign_b[:]
                        .rearrange("p (t o) -> p t o", o=1)
                        .to_broadcast([PB, SLAB, 4]),
                        op=Alu.mult,
                    )

                # ---- per-tile segment sums via one-hot matmuls (bf16 hi+lo) ----
                out_t = out_p.tile([PA, SLAB * OUTC], f32)
                for i in range(SLAB):
                    sa = sel_p.tile([PA, GP], bf16, tag="sa")
                    nc.vector.tensor_scalar(
                        sa[:], iota_t[:], mt_t[:, MSEGA + i : MSEGA + i + 1],
                        None, Alu.is_equal,
                    )
                    sb = sel_p.tile([PB, GP], bf16, tag="sb")
                    nc.vector.tensor_scalar(
                        sb[:], iota_t[0:PB, :], mt_t[0:PB, MSEGB + i : MSEGB + i + 1],
                        None, Alu.is_equal,
                    )
                    ps = ps_p.tile([GP, OUTC], f32)
                    nc.tensor.matmul(
                        ps[:], sa[:], xa_t[:, i * XCOL : i * XCOL + 90],
                        start=True, stop=False,
                    )
                    nc.tensor.matmul(
                        ps[:], sa[:], xa_t[:, i * XCOL + 91 : i * XCOL + 181],
                        start=False, stop=False,
                    )
                    nc.tensor.matmul(
                        ps[:], sb[:], xb_t[0:PB, i * XCOL : i * XCOL + 90],
                        start=False, stop=False,
                    )
                    nc.tensor.matmul(
                        ps[:], sb[:], xb_t[0:PB, i * XCOL + 91 : i * XCOL + 181],
                        start=False, stop=True,
                    )
                    nc.scalar.activation(
                        out_t[:, i * OUTC : i * OUTC + OUTC],
                        ps[:],
                        Act.Copy,
                        scale=mt_t[:, MREC + i : MREC + i + 1],
                    )

                # ---- normalize pooled quaternions ----
                o3 = out_t[:].rearrange("p (t c) -> p t c", c=OUTC)
                sq_t = sm_p.tile([PA, SLAB * 4], f32, tag="sq")
                sq3 = sq_t[:].rearrange("p (t c) -> p t c", c=4)
                nc.vector.tensor_tensor(
                    out=sq3, in0=o3[:, :, 6:10], in1=o3[:, :, 6:10], op=Alu.mult
                )
                nrm_t = sm_p.tile([PA, SLAB], f32, tag="nrm")
                nc.vector.tensor_reduce(
                    out=nrm_t[:], in_=sq3, axis=mybir.AxisListType.X, op=Alu.add
                )
                nc.scalar.sqrt(nrm_t[:], nrm_t[:])
                nc.vector.tensor_scalar(nrm_t[:], nrm_t[:], 1e-12, None, Alu.max)
                nrec_t = sm_p.tile([PA, SLAB], f32, tag="nrec")
                nc.vector.reciprocal(nrec_t[:], nrm_t[:])
                nc.vector.tensor_tensor(
                    out=o3[:, :, 6:10],
                    in0=o3[:, :, 6:10],
                    in1=nrec_t[:]
                    .rearrange("p (t o) -> p t o", o=1)
                    .to_broadcast([PA, SLAB, 4]),
                    op=Alu.mult,
                )
                nc.sync.dma_start(out=out[s], in_=out_t[:])

                # ---- voxel code decode (shift/mask on small int halves) ----
                uch = mt_t[:, MUCH : MUCH + SLAB].bitcast(i32)
                ucl = mt_t[:, MUCL : MUCL + SLAB].bitcast(i32)
                vox_t = vox_p.tile([PA, 4 * SLAB], i32)
                nc.vector.tensor_scalar(
                    vox_t[:, 0:SLAB], uch, 9, None, Alu.arith_shift_right
                )
                nc.vector.tensor_scalar(
                    vox_t[:, SLAB : 2 * SLAB], uch, 511, None, Alu.bitwise_and
                )
                nc.vector.tensor_scalar(
                    vox_t[:, 2 * SLAB : 3 * SLAB], ucl, 5, None, Alu.arith_shift_right
                )
                nc.vector.tensor_scalar(
                    vox_t[:, 3 * SLAB : 4 * SLAB], ucl, 31, None, Alu.bitwise_and
                )
                nc.scalar.dma_start(out=vox[s], in_=vox_t[:])

    nc.finalize()
    return nc


# ---------------------------------------------------------------------------
# Host-side prep / assembly
# ---------------------------------------------------------------------------
def host_prep(mu, scale, rotation, features, semantic, batch_idx, ncores=NCORES):
    n = mu.shape[0]
    idx = np.floor((mu - PC_MIN) / VOX).astype(np.int64)
    code = (
        batch_idx.astype(np.int64) * SCALE_XYZ
        + idx[:, 2] * SCALE_YZ
        + idx[:, 1] * SCALE_Z
        + idx[:, 0]
    )
    order = np.argsort(code, kind="stable")
    code_s = code[order]
    newg = np.empty(n, dtype=bool)
    newg[0] = True
    np.not_equal(code_s[1:], code_s[:-1], out=newg[1:])
    starts = np.flatnonzero(newg)
    m_used = len(starts)
    gid_s = np.cumsum(newg) - 1
    bounds = np.append(starts, n)

    t_total = -(-m_used // GP)
    tiles_per_core = -(-t_total // (ncores * SLAB)) * SLAB
    slabs = tiles_per_core // SLAB
    nt = ncores * tiles_per_core

    pf = PA + PB
    g0 = np.arange(t_total) * GP
    g1 = np.minimum(g0 + GP, m_used)
    tstart = np.full(nt, n, dtype=np.int64)
    tend = np.full(nt, n, dtype=np.int64)
    tstart[:t_total] = bounds[g0]
    tend[:t_total] = bounds[g1]
    span_max = int((tend - tstart).max())
    assert span_max <= pf, f"tile span {span_max} exceeds padded size {pf}"

    pidx = tstart[:, None] + np.arange(pf)[None, :]  # [nt, pf]
    valid = pidx < tend[:, None]
    pidx_c = np.minimum(pidx, n - 1)
    src = np.where(valid, order[pidx_c], n).ravel()
    gs = np.where(valid, gid_s[pidx_c], 0)
    seg = (gs - (np.arange(nt, dtype=np.int64) * GP)[:, None]).astype(np.float32)
    seg[~valid] = 0.0
    refsrc = np.where(valid, order[bounds[gs]], n).ravel()

    def ext(a):
        return np.vstack([a, np.zeros((1, a.shape[1]), np.float32)])

    data = np.empty((nt * pf, OUTC), dtype=np.float32)
    data[:, 0:3] = ext(mu)[src]
    data[:, 3:6] = ext(scale)[src]
    rot_e = ext(rotation)
    data[:, 6:10] = rot_e[src]
    data[:, 10:26] = ext(semantic)[src]
    data[:, 26:90] = ext(features)[src]
    hi = data.astype(BF16)
    lo = (data - hi.astype(np.float32)).astype(BF16)

    xcols = np.zeros((nt * pf, XCOL), dtype=BF16)
    xcols[:, 0:90] = hi
    xcols[:, 91:181] = lo
    xcols = xcols.reshape(nt, pf, XCOL)
    seg3 = seg.reshape(nt, pf)

    # per-group metadata
    counts = np.zeros(nt * GP, dtype=np.float32)
    counts[:m_used] = np.diff(bounds).astype(np.float32)
    rec_all = (1.0 / np.maximum(counts, 1.0)).reshape(nt, GP)
    # codes split into two small halves (device ints must stay < 2^24); pad
    # groups ship 0 and the host overwrites their voxel rows after the run
    uc_full = np.zeros(nt * GP, dtype=np.int32)
    uc_full[:m_used] = code_s[starts].astype(np.int32)
    uch_all = (uc_full >> 14).reshape(nt, GP)
    ucl_all = (uc_full & 16383).reshape(nt, GP)

    iota = np.ascontiguousarray(
        np.broadcast_to(np.arange(GP, dtype=np.float32), (PA, GP)).astype(BF16)
    )

    # quat + ref fp32 per point, tile-slot layout
    qr = np.empty((nt * pf, 8), dtype=np.float32)
    qr[:, 0:4] = rot_e[src]
    qr[:, 4:8] = rot_e[refsrc]
    qr = qr.reshape(nt, pf, 8)

    in_maps = []
    for c in range(ncores):
        sl = slice(c * tiles_per_core, (c + 1) * tiles_per_core)
        xc = xcols[sl]
        xa = (
            xc[:, :PA, :]
            .reshape(slabs, SLAB, PA, XCOL)
            .transpose(0, 2, 1, 3)
            .reshape(slabs, PA, SLAB * XCOL)
        )
        xb = (
            xc[:, PA:, :]
            .reshape(slabs, SLAB, PB, XCOL)
            .transpose(0, 2, 1, 3)
            .reshape(slabs, PB, SLAB * XCOL)
        )
        meta = np.zeros((slabs, PA, MCOL), dtype=np.float32)
        qc = qr[sl]
        meta[:, :, MQA : MQA + 64] = (
            qc[:, :PA, :]
            .reshape(slabs, SLAB, PA, 8)
            .transpose(0, 2, 1, 3)
            .reshape(slabs, PA, 64)
        )
        meta[:, :PB, MQB : MQB + 64] = (
            qc[:, PA:, :]
            .reshape(slabs, SLAB, PB, 8)
            .transpose(0, 2, 1, 3)
            .reshape(slabs, PB, 64)
        )
        meta[:, :, MREC : MREC + SLAB] = (
            rec_all[sl].reshape(slabs, SLAB, GP).transpose(0, 2, 1)
        )
        meta[:, :, MUCH : MUCH + SLAB] = np.ascontiguousarray(
            uch_all[sl].reshape(slabs, SLAB, GP).transpose(0, 2, 1)
        ).view(np.float32)
        meta[:, :, MUCL : MUCL + SLAB] = np.ascontiguousarray(
            ucl_all[sl].reshape(slabs, SLAB, GP).transpose(0, 2, 1)
        ).view(np.float32)
        meta[:, :, MSEGA : MSEGA + SLAB] = (
            seg3[sl, :PA].reshape(slabs, SLAB, PA).transpose(0, 2, 1)
        )
        meta[:, :PB, MSEGB : MSEGB + SLAB] = (
            seg3[sl, PA:].reshape(slabs, SLAB, PB).transpose(0, 2, 1)
        )
        in_maps.append(
            {
                "xa": np.ascontiguousarray(xa),
                "xb": np.ascontiguousarray(xb),
                "meta": meta,
                "iota": iota,
            }
        )
    meta_info = {
        "slabs": slabs,
        "tiles_per_core": tiles_per_core,
        "n": n,
        "m_used": m_used,
    }
    return in_maps, meta_info


def assemble(results, meta):
    n = meta["n"]
    slabs = meta["slabs"]
    rows = np.concatenate(
        [
            r["out"]
            .reshape(slabs, PA, SLAB, OUTC)
            .transpose(0, 2, 1, 3)
            .reshape(-1, OUTC)
            for r in results
        ]
    )
    voxr = np.concatenate(
        [
            r["vox"]
            .reshape(slabs, PA, 4, SLAB)
            .transpose(0, 3, 1, 2)
            .reshape(-1, 4)
            for r in results
        ]
    )
    m_dev = min(rows.shape[0], n)

    def padf(cols):
        o = np.zeros((n, cols.shape[1]), dtype=np.float32)
        o[:m_dev] = cols[:m_dev]
        return o

    mu_out = padf(rows[:, 0:3])
    sc_out = padf(rows[:, 3:6])
    rot_out = padf(rows[:, 6:10])
    sem_out = padf(rows[:, 10:26])
    feat_out = padf(rows[:, 26:90])
    m_real = min(meta["m_used"], n)
    voxel_coords = np.empty((n, 4), dtype=np.int32)
    voxel_coords[:m_real] = voxr[:m_real]
    voxel_coords[m_real:] = VOX_PAD_ROW
    return mu_out, sc_out, rot_out, feat_out, sem_out, voxel_coords


_PROGRAM_CACHE = {}


def kernel(mu, scale, rotation, features, semantic, batch_idx):
    from concourse.bass_utils import run_bass_kernel_spmd

    mu = np.asarray(mu, dtype=np.float32)
    scale = np.asarray(scale, dtype=np.float32)
    rotation = np.asarray(rotation, dtype=np.float32)
    features = np.asarray(features, dtype=np.float32)
    semantic = np.asarray(semantic, dtype=np.float32)
    batch_idx = np.asarray(batch_idx, dtype=np.int32)

    in_maps, meta = host_prep(mu, scale, rotation, features, semantic, batch_idx)
    slabs = meta["slabs"]
    if slabs not in _PROGRAM_CACHE:
        _PROGRAM_CACHE[slabs] = build_program(slabs)
    nc = _PROGRAM_CACHE[slabs]
    res = run_bass_kernel_spmd(nc, in_maps, list(range(NCORES)))
    return assemble(res.results, meta)
